# revision 1
# baseline (speedup 1.0000x reference)
"""ODE-RNN (nn_ODERNN_53987738911257) Trainium2 Bass kernel.

Strategy: data-parallel over the N=16384 sample axis across 8 NeuronCores
(2048 samples per core). The hidden state lives transposed in SBUF as
[128, 4, 2048] float32r for the whole 40-observation scan. All GEMMs run
on the tensor engine in float32r (full-rate fp32-storage matmuls). Events
are pre-scattered on the host into each core's column layout, so the
per-observation RNN update is a full-width GEMM merged under a column
mask, and the MAE loss is reduced on-chip to per-partition partials.
The final (loss, loss/tot_m) pair is assembled on the host; tot_m is a
plain sum of M and the post-loss Euler steps do not affect the output,
so neither touches the device.
"""
import sys
sys.path.insert(0, "/opt/trn_rl_repo")

import numpy as np
import ml_dtypes

import concourse.bass as bass
import concourse.tile as tile
from concourse import bacc, mybir

F32 = mybir.dt.float32
F32R = mybir.dt.float32r
BF16 = mybir.dt.bfloat16
U8 = mybir.dt.uint8
AF = mybir.ActivationFunctionType
ALU = mybir.AluOpType

P = 128
HT = 4          # hidden 512 = 4 partition tiles
CW = 512        # column chunk width
NCH = 4         # 2048 / 512
NCOLS = 2048
N_CORES = 8
NSTEPS = 40
N_SAMPLES = 16384
DT = 0.05

IB_BO1, IB_BO2, IB_BRNN, IB_BP1, IB_BP2 = 0, 4, 8, 12, 16
NB = 17


def _build_kernel(nsteps=NSTEPS, n_cores=N_CORES):
    nc = bacc.Bacc("TRN2", target_bir_lowering=False, debug=False,
                   enable_asserts=False, num_devices=n_cores)
    xt_d = nc.dram_tensor("xt", [nsteps, P, NCOLS], F32R, kind="ExternalInput")
    mt_d = nc.dram_tensor("mt", [nsteps, P, NCOLS], BF16, kind="ExternalInput")
    mk_d = nc.dram_tensor("mk", [nsteps, P, NCOLS], U8, kind="ExternalInput")
    wo1_d = nc.dram_tensor("wo1", [HT, HT, P, P], F32R, kind="ExternalInput")
    wo2_d = nc.dram_tensor("wo2", [HT, HT, P, P], F32R, kind="ExternalInput")
    whh_d = nc.dram_tensor("whh", [HT, HT, P, P], F32R, kind="ExternalInput")
    wp1_d = nc.dram_tensor("wp1", [HT, HT, P, P], F32R, kind="ExternalInput")
    wih_d = nc.dram_tensor("wih", [HT, P, P], F32R, kind="ExternalInput")
    wp2_d = nc.dram_tensor("wp2", [HT, P, P], F32R, kind="ExternalInput")
    b_d = nc.dram_tensor("bias", [P, NB], F32, kind="ExternalInput")
    loss_d = nc.dram_tensor("loss", [P, nsteps * NCH], F32,
                            kind="ExternalOutput")

    with tile.TileContext(nc) as tc:
        with (
            tc.tile_pool(name="const", bufs=1) as cpool,
            tc.tile_pool(name="stream", bufs=3) as spool,
            tc.tile_pool(name="work", bufs=12) as wpool,
            tc.tile_pool(name="psum", bufs=8, space="PSUM") as ppool,
        ):
            wo1 = cpool.tile([P, HT * HT, P], F32R, tag="wo1")
            wo2 = cpool.tile([P, HT * HT, P], F32R, tag="wo2")
            whh = cpool.tile([P, HT * HT, P], F32R, tag="whh")
            wp1 = cpool.tile([P, HT * HT, P], F32R, tag="wp1")
            wih = cpool.tile([P, HT, P], F32R, tag="wih")
            wp2 = cpool.tile([P, HT, P], F32R, tag="wp2")
            for kt in range(HT):
                for jt in range(HT):
                    nc.sync.dma_start(wo1[:, kt * HT + jt, :], wo1_d[kt, jt])
                    nc.sync.dma_start(wo2[:, kt * HT + jt, :], wo2_d[kt, jt])
                    nc.sync.dma_start(whh[:, kt * HT + jt, :], whh_d[kt, jt])
                    nc.sync.dma_start(wp1[:, kt * HT + jt, :], wp1_d[kt, jt])
                nc.sync.dma_start(wih[:, kt, :], wih_d[kt])
                nc.sync.dma_start(wp2[:, kt, :], wp2_d[kt])
            bia = cpool.tile([P, NB], F32, tag="bias")
            nc.sync.dma_start(bia[:], b_d[:])

            hT = cpool.tile([P, HT, NCOLS], F32R, tag="hT")
            loss_sb = cpool.tile([P, nsteps * NCH], F32, tag="loss")

            def bcol(i):
                return bia[:, i:i + 1]

            obs_tiles = {}

            def get_obs(k):
                if k not in obs_tiles:
                    xt = spool.tile([P, NCOLS], F32R, tag="xt")
                    nc.sync.dma_start(xt[:], xt_d[k])
                    mt = spool.tile([P, NCOLS], BF16, tag="mt")
                    nc.sync.dma_start(mt[:], mt_d[k])
                    mk = spool.tile([P, NCOLS], U8, tag="mk")
                    nc.sync.dma_start(mk[:], mk_d[k])
                    obs_tiles[k] = (xt, mt, mk)
                return obs_tiles[k]

            a1_tiles = {}

            def emit_a1(k, step, c):
                get_obs(k)
                sl = bass.ts(c, CW)
                a1s = []
                for jt in range(HT):
                    ps = ppool.tile([P, CW], F32, tag="ps")
                    for kt in range(HT):
                        nc.tensor.matmul(
                            ps[:], wo1[:, kt * HT + jt, :], hT[:, kt, sl],
                            start=(kt == 0), stop=(kt == HT - 1))
                    a1 = wpool.tile([P, CW], F32R, tag="a1")
                    nc.scalar.activation(a1[:], ps[:], AF.Tanh,
                                         bias=bcol(IB_BO1 + jt))
                    a1s.append(a1)
                a1_tiles[(step, c)] = a1s

            def emit_dh(k, step, c):
                sl = bass.ts(c, CW)
                a1s = a1_tiles.pop((step, c))
                for jt in range(HT):
                    ps = ppool.tile([P, CW], F32, tag="ps")
                    for kt in range(HT):
                        nc.tensor.matmul(
                            ps[:], wo2[:, kt * HT + jt, :], a1s[kt][:],
                            start=(kt == 0), stop=(kt == HT - 1))
                    nc.vector.scalar_tensor_tensor(
                        out=hT[:, jt, sl], in0=ps[:],
                        scalar=bcol(IB_BO2 + jt), in1=hT[:, jt, sl],
                        op0=ALU.add, op1=ALU.add)

            def emit_ob(k, c):
                xt, mt, mk = get_obs(k)
                sl = bass.ts(c, CW)
                p1s = []
                for jt in range(HT):
                    ps = ppool.tile([P, CW], F32, tag="ps")
                    for kt in range(HT):
                        nc.tensor.matmul(
                            ps[:], wp1[:, kt * HT + jt, :], hT[:, kt, sl],
                            start=(kt == 0), stop=(kt == HT - 1))
                    p1 = wpool.tile([P, CW], F32R, tag="p1")
                    nc.scalar.activation(p1[:], ps[:], AF.Relu,
                                         bias=bcol(IB_BP1 + jt))
                    p1s.append(p1)
                rps = []
                for jt in range(HT):
                    ps = ppool.tile([P, CW], F32, tag="ps")
                    nc.tensor.matmul(ps[:], wih[:, jt, :], xt[:, sl],
                                     start=True, stop=False)
                    for kt in range(HT):
                        nc.tensor.matmul(
                            ps[:], whh[:, kt * HT + jt, :], hT[:, kt, sl],
                            start=False, stop=(kt == HT - 1))
                    rps.append(ps)
                # drain the RNN psums first so their banks free before
                # the Wp2 group + loss path run
                hns = []
                for jt in range(HT):
                    hn = wpool.tile([P, CW], F32, tag="hnew")
                    nc.scalar.activation(hn[:], rps[jt][:], AF.Tanh,
                                         bias=bcol(IB_BRNN + jt))
                    hns.append(hn)
                ps_p = ppool.tile([P, CW], F32, tag="ps")
                for kt in range(HT):
                    nc.tensor.matmul(ps_p[:], wp2[:, kt, :], p1s[kt][:],
                                     start=(kt == 0), stop=(kt == HT - 1))
                for jt in range(HT):
                    nc.vector.copy_predicated(
                        hns[jt][:], mk[:, sl], hT[:, jt, sl].bitcast(F32))
                    nc.scalar.copy(hT[:, jt, sl], hns[jt][:])
                dm = wpool.tile([P, CW], F32, tag="dm")
                nc.vector.scalar_tensor_tensor(
                    out=dm[:], in0=ps_p[:], scalar=bcol(IB_BP2),
                    in1=xt[:, sl], op0=ALU.add, op1=ALU.subtract)
                nc.vector.tensor_tensor(dm[:], dm[:], mt[:, sl], ALU.mult)
                nc.vector.tensor_reduce(
                    loss_sb[:, k * NCH + c: k * NCH + c + 1], dm[:],
                    mybir.AxisListType.X, ALU.add, apply_absolute_value=True)

            def obs_phases(k):
                seq = [("a1", 1, 0), ("a1", 1, 1), ("dh", 1, 0),
                       ("a1", 1, 2), ("dh", 1, 1), ("a1", 1, 3),
                       ("dh", 1, 2), ("a1", 2, 0), ("dh", 1, 3),
                       ("a1", 2, 1), ("dh", 2, 0), ("a1", 2, 2),
                       ("dh", 2, 1), ("a1", 2, 3), ("dh", 2, 2),
                       ("ob", 0, 0), ("dh", 2, 3), ("ob", 1, 1),
                       ("ob", 2, 2), ("ob", 3, 3)]
                return [(k,) + s for s in seq]

            nc.vector.memset(hT[:].bitcast(mybir.dt.uint32), 0)
            for k in range(nsteps):
                for (kk, kind, a, b) in obs_phases(k):
                    if kind == "a1":
                        emit_a1(kk, a, b)
                    elif kind == "dh":
                        emit_dh(kk, a, b)
                    else:
                        emit_ob(kk, b)
                del obs_tiles[k]

            nc.sync.dma_start(loss_d[:], loss_sb[:])
    nc.compile()
    return nc


def _prep_inputs(X, M, batch_idx, W_ih, b_ih, W_hh, b_hh,
                 Wo1, bo1, Wo2, bo2, Wp1, bp1, Wp2, bp2):
    X = np.asarray(X, np.float32)
    M = np.asarray(M, np.float32)
    batch_idx = np.asarray(batch_idx)
    K = X.shape[0]
    npc = N_SAMPLES // N_CORES

    def wtiles(W):
        WT = np.ascontiguousarray(np.asarray(W, np.float32).T)
        ko, jo = WT.shape[0] // P, WT.shape[1] // P
        return np.ascontiguousarray(
            WT.reshape(ko, P, jo, P).transpose(0, 2, 1, 3))

    wo1 = wtiles(Wo1)
    wo2 = wtiles(np.asarray(Wo2, np.float32) * DT)
    whh = wtiles(W_hh)
    wp1 = wtiles(Wp1)
    wih = wtiles(W_ih).reshape(HT, P, P)
    wp2 = wtiles(Wp2).reshape(HT, P, P)

    bias = np.zeros((P, NB), np.float32)
    bias[:, IB_BO1:IB_BO1 + 4] = np.asarray(bo1, np.float32).reshape(4, P).T
    bias[:, IB_BO2:IB_BO2 + 4] = (np.asarray(bo2, np.float32) * DT).reshape(4, P).T
    brnn = np.asarray(b_ih, np.float32) + np.asarray(b_hh, np.float32)
    bias[:, IB_BRNN:IB_BRNN + 4] = brnn.reshape(4, P).T
    bias[:, IB_BP2] = np.asarray(bp2, np.float32)
    bias[:, IB_BP1:IB_BP1 + 4] = np.asarray(bp1, np.float32).reshape(4, P).T

    kk = np.arange(K)[:, None]
    Xs = np.zeros((K, N_SAMPLES, X.shape[2]), np.float32)
    Xs[kk, batch_idx] = X
    Ms = np.zeros((K, N_SAMPLES, X.shape[2]), np.float32)
    Ms[kk, batch_idx] = M
    obs = np.zeros((K, N_SAMPLES), np.float32)
    obs[kk, batch_idx] = 1.0

    in_maps = []
    for c in range(N_CORES):
        slc = slice(c * npc, (c + 1) * npc)
        xt = np.ascontiguousarray(Xs[:, slc].transpose(0, 2, 1))
        mtc = np.ascontiguousarray(
            Ms[:, slc].transpose(0, 2, 1)).astype(ml_dtypes.bfloat16)
        mkc = np.ascontiguousarray(np.broadcast_to(
            1.0 - obs[:, None, slc], (K, P, npc))).astype(np.uint8)
        in_maps.append({
            "xt": xt, "mt": mtc, "mk": mkc,
            "wo1": wo1, "wo2": wo2, "whh": whh, "wp1": wp1,
            "wih": wih, "wp2": wp2, "bias": bias,
        })
    tot_m = float(np.asarray(M, np.float64).sum())
    return in_maps, tot_m


class _Runner:
    """Compile once per process; re-usable across kernel() calls."""

    def __init__(self, nc, n_cores):
        import jax
        from jax.sharding import Mesh, PartitionSpec, NamedSharding
        from jax.experimental.shard_map import shard_map
        from concourse.bass2jax import (
            _bass_exec_p, install_neuronx_cc_hook, partition_id_tensor)
        install_neuronx_cc_hook()
        self.jax = jax
        self.n_cores = n_cores
        partition_name = (
            nc.partition_id_tensor.name if nc.partition_id_tensor else None)
        in_names, out_names, out_avals, zero_outs = [], [], [], []
        for alloc in nc.m.functions[0].allocations:
            if not isinstance(alloc, mybir.MemoryLocationSet):
                continue
            name = alloc.memorylocations[0].name
            if alloc.kind == "ExternalInput":
                if name != partition_name:
                    in_names.append(name)
            elif alloc.kind == "ExternalOutput":
                shape = tuple(alloc.tensor_shape)
                dtype = mybir.dt.np(alloc.dtype)
                out_names.append(name)
                out_avals.append(jax.core.ShapedArray(shape, dtype))
                zero_outs.append(np.zeros(shape, dtype))
        self.in_names = in_names
        self.out_names = out_names
        self.out_avals = out_avals
        self.zero_outs = zero_outs
        n_params = len(in_names)
        n_outs = len(out_avals)
        all_in_names = in_names + out_names
        if partition_name is not None:
            all_in_names.append(partition_name)

        def _body(*args):
            operands = list(args)
            if partition_name is not None:
                operands.append(partition_id_tensor())
            outs = _bass_exec_p.bind(
                *operands,
                out_avals=tuple(out_avals),
                in_names=tuple(all_in_names),
                out_names=tuple(out_names),
                lowering_input_output_aliases=(),
                sim_require_finite=True,
                sim_require_nnan=True,
                nc=nc,
            )
            return tuple(outs)

        devices = jax.devices()[:n_cores]
        assert len(devices) == n_cores, \
            f"need {n_cores} neuron cores, found {len(jax.devices())}"
        self.mesh = Mesh(np.asarray(devices), ("core",))
        in_specs = (PartitionSpec("core"),) * (n_params + n_outs)
        out_specs = (PartitionSpec("core"),) * n_outs
        self.fn = jax.jit(
            shard_map(_body, mesh=self.mesh, in_specs=in_specs,
                      out_specs=out_specs, check_rep=False),
            keep_unused=True)
        self.sharding = NamedSharding(self.mesh, PartitionSpec("core"))

    def run(self, in_maps):
        jax = self.jax
        devices = list(self.mesh.devices.flat)
        dev_inputs = []
        for n in self.in_names:
            shards = [jax.device_put(np.asarray(in_maps[c][n]), devices[c])
                      for c in range(self.n_cores)]
            s0 = shards[0].shape
            dev_inputs.append(jax.make_array_from_single_device_arrays(
                (self.n_cores * s0[0], *s0[1:]), self.sharding, shards))
        for z in self.zero_outs:
            shards = [jax.device_put(np.zeros(z.shape, z.dtype), devices[c])
                      for c in range(self.n_cores)]
            dev_inputs.append(jax.make_array_from_single_device_arrays(
                (self.n_cores * z.shape[0], *z.shape[1:]),
                self.sharding, shards))
        outs = self.fn(*dev_inputs)
        jax.block_until_ready(outs)
        return [
            {name: np.asarray(outs[i]).reshape(
                self.n_cores, *self.out_avals[i].shape)[c]
             for i, name in enumerate(self.out_names)}
            for c in range(self.n_cores)
        ]


_runner = None


def _get_runner():
    global _runner
    if _runner is None:
        nc = _build_kernel()
        _runner = _Runner(nc, N_CORES)
    return _runner


def kernel(X, M, batch_idx, W_ih, b_ih, W_hh, b_hh,
           Wo1, bo1, Wo2, bo2, Wp1, bp1, Wp2, bp2):
    in_maps, tot_m = _prep_inputs(
        X, M, batch_idx, W_ih, b_ih, W_hh, b_hh,
        Wo1, bo1, Wo2, bo2, Wp1, bp1, Wp2, bp2)
    results = _get_runner().run(in_maps)
    loss = sum(float(r["loss"].astype(np.float64).sum()) for r in results)
    return np.array([loss, loss / tot_m], np.float32)



# revision 2
# speedup vs baseline: 1.0657x; 1.0657x over previous
"""ODE-RNN (nn_ODERNN_53987738911257) Trainium2 Bass kernel, v2.

Data-parallel over the N=16384 sample axis across 8 NeuronCores (2048
samples per core); hidden state lives transposed in SBUF as
[128, 4, 2048] float32r for the whole 40-observation scan.

v2 reduces tensor-engine work ~2.7x vs the dense-f32r baseline:

1. z-fused Euler: with z = h@Wo1.T + bo1 (the tanh argument),
   consecutive Euler steps satisfy
     z' = z + dt * tanh(z) @ (Wo1@Wo2).T + dt * bo2@Wo1.T
   so the two Euler steps per observation need one GEMM each (A: Wo1 to
   seed z, B: the fused Wo1@Wo2 update accumulated in-place in PSUM)
   plus one GEMM (C: Wo2 on a1+a2) to reconstruct h at the observation
   -- 3 GEMM passes instead of 4.
2. fp8e4m3 DoubleRow matmuls (0.5 cycles/row, 2x fp32r rate) for every
   GEMM except A. The z PSUM holds z/dt (A's weights are Wo1/dt in
   f32r) so B's fp8 weights are the naturally-scaled Wo1@Wo2 -- no fp8
   subnormal underflow -- and the tanh reads use the Act scale=dt.
   C injects the +2*bo2 bias through an extra DoubleRow lane pair whose
   rhs is a constant all-ones fp8 lane. Numpy simulation of this exact
   quantization pipeline vs the f32 reference: rel err 6.8e-5.
3. The observation step stays dense (gpsimd gathers measured too slow
   at 21 ns/index to exploit the 25% observed-column sparsity), with
   p1/RNN/p2 GEMMs in fp8 off an fp8 shadow of h and the RNN update
   merged via predicated copy, as in the baseline.
"""
import sys
sys.path.insert(0, "/opt/trn_rl_repo")

import numpy as np
import ml_dtypes

import concourse.bass as bass
import concourse.tile as tile
from concourse import bacc, mybir

F32 = mybir.dt.float32
F32R = mybir.dt.float32r
FP8 = mybir.dt.float8e4
U8 = mybir.dt.uint8
AF = mybir.ActivationFunctionType
ALU = mybir.AluOpType
DR = mybir.MatmulPerfMode.DoubleRow
E4M3 = ml_dtypes.float8_e4m3

P = 128
HT = 4          # hidden 512 = 4 partition tiles
CW = 512        # column chunk width
NCH = 4         # 2048 / 512
NCOLS = 2048
N_CORES = 8
NSTEPS = 40
N_SAMPLES = 16384
DT = 0.05

IB_BO1, IB_BO1C, IB_BRNN, IB_BP1, IB_BP2 = 0, 4, 8, 12, 16
NB = 17


def _build_kernel(nsteps=NSTEPS, n_cores=N_CORES, reps=1):
    nc = bacc.Bacc("TRN2", target_bir_lowering=False, debug=False,
                   enable_asserts=False, num_devices=n_cores)
    xt_d = nc.dram_tensor("xt", [nsteps, P, NCOLS], F32R, kind="ExternalInput")
    x8_d = nc.dram_tensor("x8", [nsteps, P, NCOLS], FP8, kind="ExternalInput")
    mt_d = nc.dram_tensor("mt", [nsteps, P, NCOLS], F32, kind="ExternalInput")
    mo_d = nc.dram_tensor("mo", [nsteps, P, NCOLS], U8, kind="ExternalInput")
    wo1_d = nc.dram_tensor("wo1", [HT * HT, P, P], F32R, kind="ExternalInput")
    wf8_d = nc.dram_tensor("wf8", [2 * HT, P, 2 * P], FP8, kind="ExternalInput")
    wc8_d = nc.dram_tensor("wc8", [5 * HT, P, 2 * P], FP8, kind="ExternalInput")
    wr8_d = nc.dram_tensor("wr8", [3 * HT, P, 2 * P], FP8, kind="ExternalInput")
    wp18_d = nc.dram_tensor("wp18", [2 * HT, P, 2 * P], FP8,
                            kind="ExternalInput")
    wp28_d = nc.dram_tensor("wp28", [2, P, 2 * P], FP8, kind="ExternalInput")
    b_d = nc.dram_tensor("bias", [P, NB], F32, kind="ExternalInput")
    loss_d = nc.dram_tensor("loss", [P, nsteps * NCH], F32,
                            kind="ExternalOutput")

    with tile.TileContext(nc) as tc:
        with (
            tc.tile_pool(name="const", bufs=1) as cpool,
            tc.tile_pool(name="stream", bufs=3) as spool,
            tc.tile_pool(name="work", bufs=3) as wpool,
            tc.tile_pool(name="psum", bufs=8, space="PSUM") as ppool,
        ):
            wo1 = cpool.tile([P, HT * HT, P], F32R, tag="wo1")
            wf8 = cpool.tile([P, 2 * HT, 2, P], FP8, tag="wf8")
            wc8 = cpool.tile([P, 5 * HT, 2, P], FP8, tag="wc8")
            wr8 = cpool.tile([P, 3 * HT, 2, P], FP8, tag="wr8")
            wp18 = cpool.tile([P, 2 * HT, 2, P], FP8, tag="wp18")
            wp28 = cpool.tile([P, 2, 2, P], FP8, tag="wp28")
            for i in range(HT * HT):
                nc.sync.dma_start(wo1[:, i, :], wo1_d[i])
            for i in range(2 * HT):
                nc.sync.dma_start(
                    wf8[:, i, :, :].rearrange("p a b -> p (a b)"), wf8_d[i])
                nc.sync.dma_start(
                    wp18[:, i, :, :].rearrange("p a b -> p (a b)"), wp18_d[i])
            for i in range(5 * HT):
                nc.sync.dma_start(
                    wc8[:, i, :, :].rearrange("p a b -> p (a b)"), wc8_d[i])
            for i in range(3 * HT):
                nc.sync.dma_start(
                    wr8[:, i, :, :].rearrange("p a b -> p (a b)"), wr8_d[i])
            for i in range(2):
                nc.sync.dma_start(
                    wp28[:, i, :, :].rearrange("p a b -> p (a b)"), wp28_d[i])
            bia = cpool.tile([P, NB], F32, tag="bias")
            nc.sync.dma_start(bia[:], b_d[:])

            hT = cpool.tile([P, HT, NCOLS], F32R, tag="hT")
            # lanes: 0 = x8 (per-obs DMA), 1..4 = fp8 cast of hT, 5 = zeros
            h8x = cpool.tile([P, 6, NCOLS], FP8, tag="h8x")
            # lanes: 0..3 = a1, 4..7 = a2, 8 = ones (bias), 9 = zeros
            act8 = cpool.tile([P, 10, NCOLS], FP8, tag="act8")
            p18 = cpool.tile([P, HT, NCOLS], FP8, tag="p18")
            loss_sb = cpool.tile([P, nsteps * NCH], F32, tag="loss")

            def bcol(i):
                return bia[:, i:i + 1]

            obs_tiles = {}

            def get_obs(k):
                if k not in obs_tiles:
                    xt = spool.tile([P, NCOLS], F32R, tag="xt")
                    nc.sync.dma_start(xt[:], xt_d[k])
                    mt = spool.tile([P, NCOLS], F32, tag="mt")
                    nc.sync.dma_start(mt[:], mt_d[k])
                    mo = spool.tile([P, NCOLS], U8, tag="mo")
                    nc.sync.dma_start(mo[:], mo_d[k])
                    obs_tiles[k] = (xt, mt, mo)
                return obs_tiles[k]

            def emit_euler(k, c):
                sl = bass.ts(c, CW)
                # GEMM A: ps_z = (Wo1/dt)^T h   (f32r; psum holds z/dt)
                zps = []
                for jt in range(HT):
                    ps = ppool.tile([P, CW], F32, tag="ps")
                    for kt in range(HT):
                        nc.tensor.matmul(
                            ps[:], wo1[:, kt * HT + jt, :], hT[:, kt, sl],
                            start=(kt == 0), stop=False,
                            skip_group_check=True)
                    zps.append(ps)
                for jt in range(HT):
                    nc.scalar.activation(act8[:, jt, sl], zps[jt][:], AF.Tanh,
                                         bias=bcol(IB_BO1 + jt), scale=DT)
                # GEMM B: ps_z += (Wo1@Wo2)^T a1   (fp8 DoubleRow)
                for jt in range(HT):
                    for g in range(2):
                        nc.tensor.matmul(
                            zps[jt][:], wf8[:, g * HT + jt, :, :],
                            act8[:, 2 * g:2 * g + 2, sl],
                            start=False, stop=(g == 1), perf_mode=DR,
                            skip_group_check=True)
                for jt in range(HT):
                    nc.scalar.activation(act8[:, HT + jt, sl], zps[jt][:],
                                         AF.Tanh, bias=bcol(IB_BO1C + jt),
                                         scale=DT)
                # GEMM C: ps_h = Wo2^T (a1 + a2) + 2*bo2 (bias lane pair)
                for jt in range(HT):
                    ps = ppool.tile([P, CW], F32, tag="ps")
                    for l in range(5):
                        nc.tensor.matmul(
                            ps[:], wc8[:, l * HT + jt, :, :],
                            act8[:, 2 * l:2 * l + 2, sl],
                            start=(l == 0), stop=(l == 4), perf_mode=DR)
                    # h += dt * ps_h
                    nc.vector.scalar_tensor_tensor(
                        out=hT[:, jt, sl], in0=ps[:], scalar=DT,
                        in1=hT[:, jt, sl], op0=ALU.mult, op1=ALU.add)
                    # refresh the fp8 shadow of h for the obs GEMMs
                    nc.vector.tensor_scalar(
                        h8x[:, 1 + jt, sl], hT[:, jt, sl].bitcast(F32),
                        0.0, None, ALU.add)

            def emit_obs(k, c):
                xt, mt, mo = get_obs(k)
                sl = bass.ts(c, CW)
                # p1 = relu(Wp1^T h + bp1) in fp8
                pps = []
                for jt in range(HT):
                    ps = ppool.tile([P, CW], F32, tag="ps")
                    for g in range(2):
                        nc.tensor.matmul(
                            ps[:], wp18[:, g * HT + jt, :, :],
                            h8x[:, 1 + 2 * g:3 + 2 * g, sl],
                            start=(g == 0), stop=(g == 1), perf_mode=DR)
                    pps.append(ps)
                for jt in range(HT):
                    nc.scalar.activation(p18[:, jt, sl], pps[jt][:], AF.Relu,
                                         bias=bcol(IB_BP1 + jt))
                # rnn: ps_r = Wih^T x + Whh^T h  (x8 packed as lane 0)
                rps = []
                for jt in range(HT):
                    ps = ppool.tile([P, CW], F32, tag="ps")
                    for g in range(3):
                        nc.tensor.matmul(
                            ps[:], wr8[:, g * HT + jt, :, :],
                            h8x[:, 2 * g:2 * g + 2, sl],
                            start=(g == 0), stop=(g == 2), perf_mode=DR)
                    rps.append(ps)
                hns = wpool.tile([P, HT, CW], F32, tag="hns")
                for jt in range(HT):
                    nc.scalar.activation(hns[:, jt, :], rps[jt][:], AF.Tanh,
                                         bias=bcol(IB_BRNN + jt))
                # pred = Wp2^T p1 (+bp2 in the loss stt)
                ps_w = ppool.tile([P, CW], F32, tag="ps")
                for g in range(2):
                    nc.tensor.matmul(
                        ps_w[:], wp28[:, g, :, :], p18[:, 2 * g:2 * g + 2, sl],
                        start=(g == 0), stop=(g == 1), perf_mode=DR)
                dm = wpool.tile([P, CW], F32, tag="dm")
                nc.vector.scalar_tensor_tensor(
                    out=dm[:], in0=ps_w[:], scalar=bcol(IB_BP2), in1=xt[:, sl],
                    op0=ALU.add, op1=ALU.subtract)
                nc.gpsimd.tensor_tensor(dm[:], dm[:], mt[:, sl], ALU.mult)
                nc.vector.tensor_reduce(
                    loss_sb[:, k * NCH + c: k * NCH + c + 1], dm[:],
                    mybir.AxisListType.X, ALU.add, apply_absolute_value=True)
                # scatter h_new into observed columns: fill unobserved from
                # hT (mo holds 1-obs), then Act copy rounds F32 -> F32R
                for jt in range(HT):
                    nc.vector.copy_predicated(
                        hns[:, jt, :], mo[:, sl], hT[:, jt, sl].bitcast(F32))
                    nc.scalar.copy(hT[:, jt, sl], hns[:, jt, :])

            nc.vector.memset(hT[:].bitcast(mybir.dt.uint32), 0)
            nc.vector.memset(h8x[:, 1:6, :].bitcast(U8), 0)
            nc.vector.memset(act8[:, 8, :], 1.0)
            nc.vector.memset(act8[:, 9, :].bitcast(U8), 0)
            for rep in range(reps):
                for k in range(nsteps):
                    get_obs(k)
                    nc.sync.dma_start(h8x[:, 0, :], x8_d[k])
                    for c in range(NCH):
                        emit_euler(k, c)
                    for c in range(NCH):
                        emit_obs(k, c)
                    del obs_tiles[k]
                if rep + 1 < reps:
                    nc.vector.memset(hT[:].bitcast(mybir.dt.uint32), 0)
                    nc.vector.memset(h8x[:, 1:6, :].bitcast(U8), 0)

            nc.sync.dma_start(loss_d[:], loss_sb[:])
    nc.compile()
    return nc


def _wtiles(W):
    """[out, in] torch-layout weight -> [ko, jo, P, P] PE tiles of W.T."""
    WT = np.ascontiguousarray(np.asarray(W, np.float32).T)
    ko, jo = WT.shape[0] // P, WT.shape[1] // P
    return np.ascontiguousarray(WT.reshape(ko, P, jo, P).transpose(0, 2, 1, 3))


def _pair8(arr):
    """[ko, jo, P, P] (ko even) -> fp8 [ko//2 * jo, P, 2P] DoubleRow pairs."""
    ko, jo = arr.shape[0], arr.shape[1]
    out = arr.reshape(ko // 2, 2, jo, P, P).transpose(0, 2, 3, 1, 4)
    out = out.reshape(ko // 2 * jo, P, 2 * P)
    return np.ascontiguousarray(out).astype(E4M3)


def _prep_inputs(X, M, batch_idx, W_ih, b_ih, W_hh, b_hh,
                 Wo1, bo1, Wo2, bo2, Wp1, bp1, Wp2, bp2):
    X = np.asarray(X, np.float32)
    M = np.asarray(M, np.float32)
    batch_idx = np.asarray(batch_idx)
    Wo1 = np.asarray(Wo1, np.float32)
    Wo2 = np.asarray(Wo2, np.float32)
    bo1 = np.asarray(bo1, np.float32)
    bo2 = np.asarray(bo2, np.float32)
    K = X.shape[0]
    npc = N_SAMPLES // N_CORES

    wo1 = _wtiles(Wo1 / DT).reshape(HT * HT, P, P)
    wf8 = _pair8(_wtiles(Wo1 @ Wo2))
    # C weights: lanes a1 (Wo2 pairs), a2 (Wo2 pairs), bias pair:
    # lane8 weight row p==0 = 2*bo2 (rhs lane8 is all-ones), lane9 zero.
    p2 = _pair8(_wtiles(Wo2))
    bias8 = np.zeros((HT, P, 2, P), np.float32)
    for jt in range(HT):
        bias8[jt, 0, 0, :] = 2.0 * bo2[jt * P:(jt + 1) * P]
    bias8 = bias8.reshape(HT, P, 2 * P).astype(E4M3)
    wc8 = np.concatenate([p2, p2, bias8], axis=0)

    whh = _wtiles(W_hh)
    wih = _wtiles(W_ih)  # [1, HT, P, P]
    # rnn lane pairs: (wih, whh0), (whh1, whh2), (whh3, 0)
    rnn = np.zeros((3, 2, HT, P, P), np.float32)
    rnn[0, 0] = wih[0]
    rnn[0, 1] = whh[0]
    rnn[1, 0] = whh[1]
    rnn[1, 1] = whh[2]
    rnn[2, 0] = whh[3]
    wr8 = np.ascontiguousarray(
        rnn.transpose(0, 2, 3, 1, 4).reshape(3 * HT, P, 2 * P)).astype(E4M3)

    wp18 = _pair8(_wtiles(Wp1))
    wp28 = _pair8(_wtiles(Wp2))

    bias = np.zeros((P, NB), np.float32)
    bo1c = bo1 + DT * (Wo1 @ bo2)
    bias[:, IB_BO1:IB_BO1 + 4] = bo1.reshape(4, P).T
    bias[:, IB_BO1C:IB_BO1C + 4] = bo1c.reshape(4, P).T
    brnn = np.asarray(b_ih, np.float32) + np.asarray(b_hh, np.float32)
    bias[:, IB_BRNN:IB_BRNN + 4] = brnn.reshape(4, P).T
    bias[:, IB_BP1:IB_BP1 + 4] = np.asarray(bp1, np.float32).reshape(4, P).T
    bias[:, IB_BP2] = np.asarray(bp2, np.float32)

    kk = np.arange(K)[:, None]
    Xs = np.zeros((K, N_SAMPLES, X.shape[2]), np.float32)
    Xs[kk, batch_idx] = X
    Ms = np.zeros((K, N_SAMPLES, X.shape[2]), np.float32)
    Ms[kk, batch_idx] = M
    obs = np.zeros((K, N_SAMPLES), np.float32)
    obs[kk, batch_idx] = 1.0

    in_maps = []
    for c in range(N_CORES):
        slc = slice(c * npc, (c + 1) * npc)
        xt = np.ascontiguousarray(Xs[:, slc].transpose(0, 2, 1))
        mtc = np.ascontiguousarray(Ms[:, slc].transpose(0, 2, 1))
        moc = np.ascontiguousarray(np.broadcast_to(
            1.0 - obs[:, None, slc], (K, P, npc))).astype(np.uint8)
        in_maps.append({
            "xt": xt, "x8": xt.astype(E4M3), "mt": mtc, "mo": moc,
            "wo1": wo1, "wf8": wf8, "wc8": wc8, "wr8": wr8,
            "wp18": wp18, "wp28": wp28, "bias": bias,
        })
    tot_m = float(np.asarray(M, np.float64).sum())
    return in_maps, tot_m


class _Runner:
    """Compile once per process; re-usable across kernel() calls."""

    def __init__(self, nc, n_cores):
        import jax
        from jax.sharding import Mesh, PartitionSpec, NamedSharding
        from jax.experimental.shard_map import shard_map
        from concourse.bass2jax import (
            _bass_exec_p, install_neuronx_cc_hook, partition_id_tensor)
        install_neuronx_cc_hook()
        self.jax = jax
        self.n_cores = n_cores
        partition_name = (
            nc.partition_id_tensor.name if nc.partition_id_tensor else None)
        in_names, out_names, out_avals, zero_outs = [], [], [], []
        for alloc in nc.m.functions[0].allocations:
            if not isinstance(alloc, mybir.MemoryLocationSet):
                continue
            name = alloc.memorylocations[0].name
            if alloc.kind == "ExternalInput":
                if name != partition_name:
                    in_names.append(name)
            elif alloc.kind == "ExternalOutput":
                shape = tuple(alloc.tensor_shape)
                dtype = mybir.dt.np(alloc.dtype)
                out_names.append(name)
                out_avals.append(jax.core.ShapedArray(shape, dtype))
                zero_outs.append(np.zeros(shape, dtype))
        self.in_names = in_names
        self.out_names = out_names
        self.out_avals = out_avals
        self.zero_outs = zero_outs
        n_params = len(in_names)
        n_outs = len(out_avals)
        all_in_names = in_names + out_names
        if partition_name is not None:
            all_in_names.append(partition_name)

        def _body(*args):
            operands = list(args)
            if partition_name is not None:
                operands.append(partition_id_tensor())
            outs = _bass_exec_p.bind(
                *operands,
                out_avals=tuple(out_avals),
                in_names=tuple(all_in_names),
                out_names=tuple(out_names),
                lowering_input_output_aliases=(),
                sim_require_finite=True,
                sim_require_nnan=True,
                nc=nc,
            )
            return tuple(outs)

        devices = jax.devices()[:n_cores]
        assert len(devices) == n_cores, \
            f"need {n_cores} neuron cores, found {len(jax.devices())}"
        self.mesh = Mesh(np.asarray(devices), ("core",))
        in_specs = (PartitionSpec("core"),) * (n_params + n_outs)
        out_specs = (PartitionSpec("core"),) * n_outs
        self.fn = jax.jit(
            shard_map(_body, mesh=self.mesh, in_specs=in_specs,
                      out_specs=out_specs, check_rep=False),
            keep_unused=True)
        self.sharding = NamedSharding(self.mesh, PartitionSpec("core"))

    def run(self, in_maps):
        jax = self.jax
        devices = list(self.mesh.devices.flat)
        dev_inputs = []
        for n in self.in_names:
            shards = [jax.device_put(np.asarray(in_maps[c][n]), devices[c])
                      for c in range(self.n_cores)]
            s0 = shards[0].shape
            dev_inputs.append(jax.make_array_from_single_device_arrays(
                (self.n_cores * s0[0], *s0[1:]), self.sharding, shards))
        for z in self.zero_outs:
            shards = [jax.device_put(np.zeros(z.shape, z.dtype), devices[c])
                      for c in range(self.n_cores)]
            dev_inputs.append(jax.make_array_from_single_device_arrays(
                (self.n_cores * z.shape[0], *z.shape[1:]),
                self.sharding, shards))
        outs = self.fn(*dev_inputs)
        jax.block_until_ready(outs)
        return [
            {name: np.asarray(outs[i]).reshape(
                self.n_cores, *self.out_avals[i].shape)[c]
             for i, name in enumerate(self.out_names)}
            for c in range(self.n_cores)
        ]


_runner = None


def _get_runner():
    global _runner
    if _runner is None:
        nc = _build_kernel()
        _runner = _Runner(nc, N_CORES)
    return _runner


def kernel(X, M, batch_idx, W_ih, b_ih, W_hh, b_hh,
           Wo1, bo1, Wo2, bo2, Wp1, bp1, Wp2, bp2):
    in_maps, tot_m = _prep_inputs(
        X, M, batch_idx, W_ih, b_ih, W_hh, b_hh,
        Wo1, bo1, Wo2, bo2, Wp1, bp1, Wp2, bp2)
    results = _get_runner().run(in_maps)
    loss = sum(float(r["loss"].astype(np.float64).sum()) for r in results)
    return np.array([loss, loss / tot_m], np.float32)
